# revision 35
# baseline (speedup 1.0000x reference)
"""Causal self-attention on Trainium2, tensor-parallel over heads across 8 NeuronCores.

Strategy (sharding_hint "tensor-parallel split the n_heads axis"):
  - Each core c owns heads {2c, 2c+1} == columns [128c, 128c+128) of Wq/Wk/Wv
    and rows [128c, 128c+128) of Wo.
  - Per core: QT/KT = (x @ W{q,k})^T in [feat, tok] layout (both heads stacked
    on the 128 partitions: h0 rows 0-63, h1 rows 64-127).  V in [tok, feat]
    layout with an appended ones column per head (denominator trick, 65-wide
    blocks).
  - Scores computed transposed ([k, q]) so exp(scoresT) feeds the PV matmul
    directly; row 64 of the PV output is the softmax denominator.
  - The two heads' K=64 score matmuls are emitted back-to-back into the two
    PSUM banks of one [128, 2*WIN] tile and released by a single shared exp:
    the PE dual-issues them in the 64x128 row-tile pair (T0/T8), doubling
    score throughput (measured 114ns/matmul vs 221ns serial for N=512).
  - One exp per k-tile covers both heads via a strided [128, 2, N] AP, so
    both PV matmuls (and the next score pair) unblock simultaneously.
  - Normalization: 1/denominator via DVE reciprocal, replicated over the 64
    head rows with a GpSimd partition_broadcast, folded into the PV PSUM ->
    aoT evacuation multiply.  No PE broadcast matmuls, no ln/exp reciprocal.
  - Partial out-projection y_c = attnout_c @ Wo_c; host sums the 8 partials
    and adds bo + bv @ Wo (V-bias is a rank-1 term, folded on host).
  - bq/bk are folded into the PSUM->SBUF evacuation (per-partition
    tensor_scalar add), so no bias matmuls run on the PE.

Schedule: x is loaded window-major (one DMA per batch x 512-token window, all
in flight from t=0; weights stream on the scalar-engine DGE in parallel), so
the first QKV matmul starts early.  A single global filler deque (batch-1 QKV
chunks, rolling aoT-evac + out-proj thunks) is drained one thunk per k-tile,
keeping PE demand high so the HAM clock gate stays at 8/8; out-proj PSUM
evacuations round-robin across Vector/GpSimd/Scalar so no single evac engine
backs the PE up.  Matmul inputs are bf16 (PSUM fp32).
"""

import sys

if "/opt/trn_rl_repo" not in sys.path:
    sys.path.insert(0, "/opt/trn_rl_repo")

from contextlib import ExitStack

import ml_dtypes
import numpy as np

import concourse.bass as bass
import concourse.mybir as mybir
import concourse.tile as tile

F32 = mybir.dt.float32
BF = mybir.dt.bfloat16
NPBF = ml_dtypes.bfloat16
EXP = mybir.ActivationFunctionType.Exp

P = 128  # partition tile
HD = 64  # head dim
HC = 2  # heads per core (HC*HD == P)
WIN = 512  # token window (one PSUM bank of fp32)
VB = HD + 1  # v block: [V(64) | ones]
VW = HC * VB  # v cols per token tile
N_WARM = 4  # PE warm-up matmuls (run under the first x-window DMA shadow)


def _legalize_waits(nc):
    """This walrus build encodes at most ONE semaphore wait per instruction
    (setupSyncWait raises "Too many sync wait commands" otherwise).  Tile
    freely emits 2+ waits, so excess waits are moved onto injected same-engine
    NoOps (one wait each) directly before the instruction."""
    nop_id = 0
    for fn in nc.m.functions:
        for blk in fn.blocks:
            out = []
            for inst in blk.instructions:
                if type(inst).__name__ != "InstNoOp":
                    si = inst.sync_info
                    waits = list(si.on_wait or []) if si is not None else []
                    if len(waits) > 1:
                        for w in waits[1:]:
                            nop = mybir.InstNoOp(
                                name=f"nopw-{nop_id}",
                                engine=inst.engine,
                                ins=[],
                                outs=[],
                                sync_info=mybir.SyncInfo(on_wait=[w], on_update=[]),
                            )
                            nop_id += 1
                            out.append(nop)
                        si.on_wait = waits[:1]
                out.append(inst)
            blk.instructions[:] = out


def build_nc(B, T, D, n_cores, legalize=True):
    nj = D // P  # contraction tiles for projections
    n_win = T // WIN  # q windows per batch
    n_qt = T // P  # token tiles per batch
    tpw = n_qt // n_win  # token tiles per window
    M = B * T

    nc = bass.Bass("TRN2", target_bir_lowering=False, debug=False, num_devices=n_cores)

    xt = nc.dram_tensor("xt", [D, M], BF, kind="ExternalInput").ap()
    wq = nc.dram_tensor("wq", [P, D], BF, kind="ExternalInput").ap()
    wk = nc.dram_tensor("wk", [P, D], BF, kind="ExternalInput").ap()
    wv = nc.dram_tensor("wv", [P, D], BF, kind="ExternalInput").ap()
    wo = nc.dram_tensor("wo", [P, D], BF, kind="ExternalInput").ap()
    bq = nc.dram_tensor("bq", [P, 1], F32, kind="ExternalInput").ap()
    bk = nc.dram_tensor("bk", [P, 1], F32, kind="ExternalInput").ap()
    msk = nc.dram_tensor("msk", [P, P], BF, kind="ExternalInput").ap()
    y = nc.dram_tensor("y", [M, D], BF, kind="ExternalOutput").ap()

    with tile.TileContext(nc) as tc, ExitStack() as ctx:
        const = ctx.enter_context(tc.tile_pool(name="const", bufs=1))
        xbp = ctx.enter_context(tc.tile_pool(name="xb", bufs=2))
        qkp = ctx.enter_context(tc.tile_pool(name="qk", bufs=2))
        vp = ctx.enter_context(tc.tile_pool(name="vaug", bufs=2))
        atp = ctx.enter_context(tc.tile_pool(name="attnT", bufs=8))
        aop = ctx.enter_context(tc.tile_pool(name="aoT", bufs=2))
        rcp = ctx.enter_context(tc.tile_pool(name="rc", bufs=2))
        psbp = ctx.enter_context(tc.tile_pool(name="psbs", bufs=2))
        pvrp = ctx.enter_context(tc.tile_pool(name="pvraw", bufs=4))
        yp = ctx.enter_context(tc.tile_pool(name="ysb", bufs=6))
        ps_sc = ctx.enter_context(tc.tile_pool(name="ps_sc", bufs=2, space="PSUM"))
        ps_pv = ctx.enter_context(tc.tile_pool(name="ps_pv", bufs=2, space="PSUM"))
        ps_proj = ctx.enter_context(tc.tile_pool(name="ps_proj", bufs=2, space="PSUM"))

        wq_s = const.tile([P, D], BF, tag="wq")
        wk_s = const.tile([P, D], BF, tag="wk")
        wv_s = const.tile([P, D], BF, tag="wv")
        wo_s = const.tile([P, D], BF, tag="wo")
        bq_s = const.tile([P, 1], F32, tag="bq")
        bk_s = const.tile([P, 1], F32, tag="bk")
        msk_s = const.tile([P, P], BF, tag="msk")
        warm_s = const.tile([P, WIN], BF, tag="warm")
        ones_s = const.tile([P, HD], BF, tag="ones")
        nc.vector.memset(warm_s[:, :], 1.0)
        nc.vector.memset(ones_s[:, :], 1.0)

        # x arrives window-major: one DMA per (batch, window) covering all nj
        # feature tiles, so window w's projections unblock after ~1MB.
        xbs = {}
        for b in range(B):
            xbs[b] = xbp.tile([P, n_win * nj * WIN], BF, tag="xb", name=f"xb{b}")

        def x_dma(b, w, split=False):
            src = xt[:, b * T + w * WIN : b * T + (w + 1) * WIN].rearrange(
                "(j p) c -> p j c", p=P
            )
            dst = xbs[b][:, w * nj * WIN : (w + 1) * nj * WIN].rearrange(
                "p (j c) -> p j c", j=nj
            )
            if split:
                # first windows: halve the arrival latency by using two
                # DMA queues so the first projection starts ~1.5us earlier
                h = nj // 2
                nc.sync.dma_start(dst[:, 0:h], src[:, 0:h])
                nc.gpsimd.dma_start(dst[:, h:nj], src[:, h:nj])
            else:
                nc.sync.dma_start(dst, src)

        # q/k weights + first x window first; everything else behind them.
        nc.scalar.dma_start(wq_s[:, :], wq[:, :])
        x_dma(0, 0, split=True)
        nc.scalar.dma_start(wk_s[:, :], wk[:, :])
        nc.scalar.dma_start(bq_s[:, :], bq[:, :])
        nc.scalar.dma_start(bk_s[:, :], bk[:, :])
        x_dma(0, 1, split=True)
        nc.scalar.dma_start(wv_s[:, :], wv[:, :])
        nc.scalar.dma_start(msk_s[:, :], msk[:, :])
        x_dma(0, 2)
        x_dma(0, 3)
        nc.scalar.dma_start(wo_s[:, :], wo[:, :])
        for w in range(n_win):
            x_dma(1, w)

        # PE warm-up under the first x DMA's shadow.
        psw = ps_proj.tile([P, WIN], F32, tag="proj")
        for i in range(N_WARM):
            nc.tensor.matmul(
                psw[:, :], warm_s[:, 0:P], warm_s[:, :], start=True, stop=True
            )

        st = {}

        def xsl(b, w, j, off=0, n=WIN):
            base = (w * nj + j) * WIN + off
            return xbs[b][:, base : base + n]

        def alloc_batch(b):
            st[b] = {
                "qt": qkp.tile([P, T], BF, tag="qt", name=f"qt{b}"),
                "kt": qkp.tile([P, T], BF, tag="kt", name=f"kt{b}"),
                "vaug": vp.tile([P, n_qt * VW], BF, tag="vaug", name=f"vaug{b}"),
                "aoT": aop.tile([P, T], BF, tag="aoT", name=f"aoT{b}"),
                "vready": False,
            }

        def proj_chunk(b, w, which):
            s = st[b]
            ws = w * WIN
            w_s, b_s, dst = (
                (wq_s, bq_s, s["qt"]) if which == "q" else (wk_s, bk_s, s["kt"])
            )
            psp = ps_proj.tile([P, WIN], F32, tag="proj", name=f"ps{which}{b}_{w}")
            for j in range(nj):
                nc.tensor.matmul(
                    psp[:, :],
                    w_s[:, j * P : (j + 1) * P],
                    xsl(b, w, j),
                    start=(j == 0),
                    stop=(j == nj - 1),
                )
            # evac both heads in one DVE op, bias folded in (per-partition)
            nc.vector.tensor_scalar_add(
                dst[:, ws : ws + WIN], psp[:, :], b_s[:, 0:1]
            )

        def v_tile(b, t):
            s = st[b]
            if not s["vready"]:
                va4 = s["vaug"].rearrange("p (t s c) -> p t s c", s=HC, c=VB)
                nc.vector.memset(va4[:, :, :, HD : HD + 1], 1.0)  # ones col
                s["vready"] = True
            w, sub = t // tpw, t % tpw
            psv = ps_proj.tile([P, P], F32, tag="proj", name=f"psv{b}_{t}")
            for j in range(nj):
                nc.tensor.matmul(
                    psv[:, :],
                    xsl(b, w, j, sub * P, P),
                    wv_s[:, j * P : (j + 1) * P],
                    start=(j == 0),
                    stop=(j == nj - 1),
                )
            va4 = s["vaug"].rearrange("p (t s c) -> p t s c", s=HC, c=VB)
            nc.vector.tensor_copy(
                va4[:, t, :, 0:HD], psv[:, :].rearrange("p (s c) -> p s c", s=HC)
            )

        def qkv_thunks(b, w):
            th = [
                lambda b=b, w=w: proj_chunk(b, w, "q"),
                lambda b=b, w=w: proj_chunk(b, w, "k"),
            ]
            for t in range(w * tpw, (w + 1) * tpw):
                th.append(lambda b=b, t=t: v_tile(b, t))
            return th

        # out-proj PSUM evacuation round-robins across the three non-PE
        # compute engines so no single engine's backlog stalls the PE.
        evac_rr = [0]

        def outproj_tile(b, t):
            toff = b * T
            aoT = st[b]["aoT"]
            ysb = yp.tile([P, D], BF, tag="ysb", name=f"ysb{b}_{t}")
            for ui, u0 in enumerate(range(0, D, WIN)):
                psy = ps_proj.tile([P, WIN], F32, tag="proj", name=f"psy{b}_{t}_{ui}")
                nc.tensor.matmul(
                    psy[:, :],
                    aoT[:, t * P : (t + 1) * P],
                    wo_s[:, u0 : u0 + WIN],
                    start=True,
                    stop=True,
                )
                if tail_mode[0]:
                    # attention is over: DVE and Act are both idle; alternate
                    # so the two halves evacuate in parallel.
                    if ui == 0:
                        nc.scalar.copy(ysb[:, u0 : u0 + WIN], psy[:, :])
                    else:
                        nc.vector.tensor_copy(ysb[:, u0 : u0 + WIN], psy[:, :])
                else:
                    # DVE-only during attention: the Activation engine must
                    # stay dedicated to the exp stream that paces PV.
                    nc.vector.tensor_copy(ysb[:, u0 : u0 + WIN], psy[:, :])
            if tail_mode[0]:
                # spread tail DMA issue across idle engine queues
                r = evac_rr[0] = (evac_rr[0] + 1) % 3
                eng = (nc.sync, nc.gpsimd, nc.sync)[r]
                eng.dma_start(y[toff + t * P : toff + (t + 1) * P, :], ysb[:, :])
            else:
                nc.sync.dma_start(y[toff + t * P : toff + (t + 1) * P, :], ysb[:, :])

        # Global filler deque of (deadline_key, thunk).  Deadline keys are
        # global window indices (b*n_win+w) for thunks that MUST trace before
        # that attention window; soft thunks (out-proj) use 99.
        dq = []
        tail_mode = [False]
        # Filler reserve: keep ~2 thunks per remaining attention window so
        # the late windows (whose own out-proj cannot exist yet) still have
        # PE filler work -- otherwise the HAM clock gate halves the clock
        # right when the serial tail begins.
        reserve = [0]

        def pop_fill(n=1):
            if len(dq) > reserve[0] + 6:
                n += 1
            for _ in range(n):
                if len(dq) > reserve[0]:
                    dq.pop(0)[1]()

        def force_drain(gwi):
            # pop only entries whose deadline is due, preserving the relative
            # order of soft thunks (aoT evac stays ahead of its out-proj).
            i = 0
            while i < len(dq):
                if dq[i][0] <= gwi:
                    dq.pop(i)[1]()
                else:
                    i += 1

        def recip_phase(b, pspv, ws):
            # 1/d = exp(-ln(d)) on the Activation engine (InstReciprocal on
            # DVE costs ~4us/call and custom-DVE ISA ops fail this walrus
            # build's codegen).  Emitted at window end; the PE-side broadcast
            # is emitted a few filler thunks LATER so it never blocks the
            # in-order PE queue while this Act chain completes.
            lg = rcp.tile([P, WIN], F32, tag="lg", name=f"lg{b}_{ws}")
            rc2 = rcp.tile([P, WIN], BF, tag="rc", name=f"rc{b}_{ws}")
            LN = mybir.ActivationFunctionType.Ln
            for h in range(HC):
                nc.scalar.activation(
                    lg[h * HD : h * HD + 1, :], pspv[h][HD : HD + 1, :], LN
                )
                nc.scalar.activation(
                    rc2[h * HD : h * HD + 1, :],
                    lg[h * HD : h * HD + 1, :],
                    EXP,
                    scale=-1.0,
                )
            return rc2

        def aot_evac(b, pspv, rc2, ws):
            # normalized attn-out: aoT[h] = pspv[h][0:64] * (1/denominator).
            # 1/d sits on rows 0/64 of rc2 so the two K=1 broadcast matmuls
            # occupy disjoint 64x64 PE quadrants (dual-issued); the
            # row-replicated 1/d is staged to SBUF by the Activation engine
            # (one copy covers both heads) and the per-head multiplies run on
            # DVE (PSUM x SBUF -> SBUF bf16).
            psb = ps_proj.tile([P, WIN], F32, tag="proj", name=f"psb{b}_{ws}")
            for h in range(HC):
                nc.tensor.matmul(
                    psb[h * HD : (h + 1) * HD, :],
                    ones_s[h * HD : h * HD + 1, 0:HD],
                    rc2[h * HD : h * HD + 1, :],
                    start=True,
                    stop=True,
                )
            psbs = psbp.tile([P, WIN], BF, tag="psbs", name=f"psbs{b}_{ws}")
            nc.scalar.copy(psbs[:, :], psb[:, :])
            for h in range(HC):
                nc.vector.tensor_mul(
                    st[b]["aoT"][h * HD : (h + 1) * HD, ws : ws + WIN],
                    pspv[h][0:HD, :],
                    psbs[h * HD : (h + 1) * HD, :],
                )

        def attn_window(b, w):
            # Per k-tile j: both heads' K=64 score matmuls go back-to-back
            # into the two banks of one [128, 2*WIN] PSUM tile -> the PE
            # dual-issues them as a 64x128 row-tile pair.  One exp over a
            # strided [128, 2, N] AP covers both heads, so the pair of PV
            # matmuls (and the next window's score pair) release together.
            # PV for k-tile j is traced after the scores of j+1 so the PE
            # never waits on the exp; one filler thunk per k-tile keeps PE
            # demand (and the HAM clock) up.
            s = st[b]
            qt_s, kt_s, vaug = s["qt"], s["kt"], s["vaug"]
            ws = w * WIN
            njt = (ws + WIN) // P  # causal k tiles for this window
            pspv = [
                ps_pv.tile([VB, WIN], F32, tag="pv", name=f"pspv{b}_{w}_{_h}")
                for _h in range(HC)
            ]

            def flush_pv(at3, j, N, qoff):
                for h in range(HC):
                    nc.tensor.matmul(
                        pspv[h][:, qoff:WIN],
                        vaug[:, j * VW + h * VB : j * VW + h * VB + VB],
                        at3[:, h, 0:N],
                        start=(j == 0),
                        stop=(j == njt - 1),
                    )

            # PV lags the scores by TWO k-tiles: every cross-engine dep
            # (exp, mask, previous window's evac muls for the j==0 start)
            # is ~2us stale by the time the PE's in-order queue reaches the
            # PV matmul, so it never blocks the instructions behind it.
            pend = []
            for j in range(njt):
                qstart = max(ws, j * P)
                N = ws + WIN - qstart
                pss = ps_sc.tile([P, HC * WIN], F32, tag="sc", name=f"pss{b}_{w}_{j}")
                pss3 = pss.rearrange("p (h c) -> p h c", h=HC)
                if not dq:
                    # keep the HAM clock gate seeing PE activity when the
                    # filler stream runs dry
                    nc.tensor.matmul(
                        psw[:, :], warm_s[:, 0:P], warm_s[:, :], start=True, stop=True
                    )
                # the dual-issue pair: adjacent, disjoint row groups + banks
                for h in range(HC):
                    nc.tensor.matmul(
                        pss3[:, h, 0:N],
                        kt_s[h * HD : (h + 1) * HD, j * P : (j + 1) * P],
                        qt_s[h * HD : (h + 1) * HD, qstart : qstart + N],
                        start=True,
                        stop=True,
                    )
                at = atp.tile([P, HC * WIN], BF, tag="at", name=f"at{b}_{w}_{j}")
                at3 = at.rearrange("p (h c) -> p h c", h=HC)
                if N == WIN:
                    # contiguous 2D AP: the strided 3-dim form costs ~34% more
                    nc.scalar.activation(at[:, :], pss[:, :], EXP)
                else:
                    nc.scalar.activation(at3[:, :, 0:N], pss3[:, :, 0:N], EXP)
                if j * P >= ws:  # zero the upper triangle post-exp
                    for h in range(HC):
                        nc.gpsimd.tensor_mul(
                            at3[:, h, 0:P], at3[:, h, 0:P], msk_s[:, :]
                        )
                pend.append((at3, j, N, qstart - ws))
                if len(pend) > 2:
                    flush_pv(*pend.pop(0))
                    pop_fill(1)
            while pend:
                flush_pv(*pend.pop(0))
                pop_fill(1)
            # the Act-engine reciprocal chain overlaps the PE work below; the
            # PE-side broadcast only comes after it (in aot_evac), so the
            # in-order PE queue never stalls on the ~2us Act round-trip.
            # Coverage here is MANDATORY (ignore the reserve): without it the
            # broadcast matmul blocks the queue and the HAM clock halves.
            rc2 = recip_phase(b, pspv, ws)
            popped = 0
            while popped < 2 and dq:
                dq.pop(0)[1]()
                popped += 1
            for _ in range(0 if popped >= 2 else 4):
                nc.tensor.matmul(
                    psw[:, :], warm_s[:, 0:P], warm_s[:, :], start=True, stop=True
                )
            aot_evac(b, pspv, rc2, ws)
            # denominators: reciprocal (DVE) + row-broadcast (GpSimd) feed the
            # PV evacuation multiply; evacs are deadline thunks (must trace
            # before the next window reuses the PV PSUM banks), out-proj for
            # this window's tiles follows them in the deque as soft fillers.
            for t in range(w * tpw, (w + 1) * tpw):
                dq.append((99, lambda b=b, t=t: outproj_tile(b, t)))

        # ---- schedule ----
        # Only b0/w0's QKV runs eagerly so window-0 attention (and the Act
        # engine's exp stream) starts as early as possible; all remaining QKV
        # is deferred into the filler stream with per-window deadlines so the
        # whole kernel keeps enough PE demand to hold the HAM clock at 8/8.
        alloc_batch(0)
        alloc_batch(1)
        for f in qkv_thunks(0, 0):
            f()
        for w in range(1, n_win):
            dq.extend((w, f) for f in qkv_thunks(0, w))
        for w in (0, 1):
            dq.extend((n_win + w, f) for f in qkv_thunks(1, w))
        for b in range(B):
            for w in range(n_win):
                gwi = b * n_win + w
                reserve[0] = min(8, 2 * (B * n_win - gwi - 1))
                force_drain(gwi)
                attn_window(b, w)
                if (b, w) == (0, 2):
                    dq.extend((n_win + 2, f) for f in qkv_thunks(1, 2))
                if (b, w) == (0, 3):
                    dq.extend((n_win + 3, f) for f in qkv_thunks(1, 3))
        tail_mode[0] = True
        while dq:
            dq.pop(0)[1]()

    if legalize:
        _legalize_waits(nc)
    return nc


def make_in_maps(x, Wq, bq, Wk, bk, Wv, Wo, n_cores):
    x = np.asarray(x, dtype=np.float32)
    Bb, Tt, Dd = x.shape
    M = Bb * Tt
    xt = np.ascontiguousarray(x.reshape(M, Dd).T.astype(NPBF))
    mask = np.where(
        np.arange(P)[:, None] > np.arange(P)[None, :], 0.0, 1.0
    ).astype(NPBF)

    def wslice(W, c, scale=1.0):
        Wc = np.asarray(W, np.float32)[:, c * P : (c + 1) * P] * np.float32(scale)
        return np.ascontiguousarray(
            Wc.reshape(Dd // P, P, P).transpose(1, 0, 2).reshape(P, Dd).astype(NPBF)
        )

    qscale = 1.0 / np.sqrt(HD)
    in_maps = []
    for c in range(n_cores):
        cs = slice(c * P, (c + 1) * P)
        in_maps.append(
            {
                "xt": xt,
                "wq": wslice(Wq, c, qscale),
                "wk": wslice(Wk, c),
                "wv": wslice(Wv, c),
                "wo": np.ascontiguousarray(
                    np.asarray(Wo, np.float32)[cs, :].astype(NPBF)
                ),
                "bq": np.ascontiguousarray(
                    (np.asarray(bq, np.float32)[cs] * np.float32(qscale)).reshape(
                        P, 1
                    )
                ),
                "bk": np.ascontiguousarray(
                    np.asarray(bk, np.float32)[cs].reshape(P, 1)
                ),
                "msk": mask,
            }
        )
    return in_maps


_NC_CACHE = {}


def get_nc(B, T, D, n_cores):
    key = (B, T, D, n_cores)
    if key not in _NC_CACHE:
        _NC_CACHE[key] = build_nc(B, T, D, n_cores)
    return _NC_CACHE[key]


def kernel(**inputs):
    from concourse.bass_utils import run_bass_kernel_spmd

    x = np.asarray(inputs["x"], np.float32)
    Bb, Tt, Dd = x.shape
    n_cores = 8
    nc = get_nc(Bb, Tt, Dd, n_cores)
    in_maps = make_in_maps(
        x,
        inputs["Wq"],
        inputs["bq"],
        inputs["Wk"],
        inputs["bk"],
        inputs["Wv"],
        inputs["Wo"],
        n_cores,
    )
    res = run_bass_kernel_spmd(nc, in_maps, core_ids=list(range(n_cores)))
    y = np.zeros((Bb * Tt, Dd), dtype=np.float64)
    for r in res.results:
        y += r["y"].astype(np.float64)
    # V-bias is rank-1 through Wo; fold it (and bo) on the host.
    y += (
        np.asarray(inputs["bv"], np.float64) @ np.asarray(inputs["Wo"], np.float64)
        + np.asarray(inputs["bo"], np.float64)
    )[None, :]
    return y.reshape(Bb, Tt, Dd).astype(np.float32)


# revision 37
# speedup vs baseline: 1.0089x; 1.0089x over previous
"""Causal self-attention on Trainium2, tensor-parallel over heads across 8 NeuronCores.

Strategy (sharding_hint "tensor-parallel split the n_heads axis"):
  - Each core c owns heads {2c, 2c+1} == columns [128c, 128c+128) of Wq/Wk/Wv
    and rows [128c, 128c+128) of Wo.
  - Per core: QT/KT = (x @ W{q,k})^T in [feat, tok] layout (both heads stacked
    on the 128 partitions: h0 rows 0-63, h1 rows 64-127).  V in [tok, feat]
    layout with an appended ones column per head (denominator trick, 65-wide
    blocks).
  - Scores computed transposed ([k, q]) so exp(scoresT) feeds the PV matmul
    directly; row 64 of the PV output is the softmax denominator.
  - The two heads' K=64 score matmuls are emitted back-to-back into the two
    PSUM banks of one [128, 2*WIN] tile and released by a single shared exp:
    the PE dual-issues them in the 64x128 row-tile pair (T0/T8), doubling
    score throughput (measured 114ns/matmul vs 221ns serial for N=512).
  - One exp per k-tile covers both heads via a strided [128, 2, N] AP, so
    both PV matmuls (and the next score pair) unblock simultaneously.
  - Normalization: 1/denominator = exp(-ln(d)) on the Activation engine
    (bf16), replicated over the 64 head rows by a pair of K=1 broadcast
    matmuls in disjoint 64x64 PE quadrants, staged to SBUF by one Act copy,
    and folded into the PV PSUM -> aoT evacuation multiply on DVE.
  - Partial out-projection y_c = attnout_c @ Wo_c; host sums the 8 partials
    and adds bo + bv @ Wo (V-bias is a rank-1 term, folded on host).
  - bq/bk are folded into the PSUM->SBUF evacuation (per-partition
    tensor_scalar add), so no bias matmuls run on the PE.

Schedule: x is loaded window-major (one DMA per batch x 512-token window, all
in flight from t=0; weights stream on the scalar-engine DGE in parallel), so
the first QKV matmul starts early.  PV lags the scores by two k-tiles so its
cross-engine deps (exp, mask, previous window's evac) are stale when the
in-order PE queue reaches it.  A single global filler deque (deferred QKV
chunks + out-proj thunks, with per-window deadline draining and a reserve
that saves fillers for the late windows) is popped one thunk per k-tile; at
each window boundary the Act-side reciprocal chain is emitted BEFORE two
mandatory filler pops (warm matmuls as backstop) so the PE never stalls on
it -- an exposed PE idle of ~1.7us makes the HAM power governor halve the
clock for ~7-10us.  Matmul inputs are bf16 (PSUM fp32).
"""

import sys

if "/opt/trn_rl_repo" not in sys.path:
    sys.path.insert(0, "/opt/trn_rl_repo")

from contextlib import ExitStack

import ml_dtypes
import numpy as np

import concourse.bass as bass
import concourse.mybir as mybir
import concourse.tile as tile

F32 = mybir.dt.float32
BF = mybir.dt.bfloat16
NPBF = ml_dtypes.bfloat16
EXP = mybir.ActivationFunctionType.Exp

P = 128  # partition tile
HD = 64  # head dim
HC = 2  # heads per core (HC*HD == P)
WIN = 512  # token window (one PSUM bank of fp32)
VB = HD + 1  # v block: [V(64) | ones]
VW = HC * VB  # v cols per token tile
N_WARM = 6  # PE warm-up matmuls (run under the first x-window DMA shadow)


def _legalize_waits(nc):
    """This walrus build encodes at most ONE semaphore wait per instruction
    (setupSyncWait raises "Too many sync wait commands" otherwise).  Tile
    freely emits 2+ waits, so excess waits are moved onto injected same-engine
    NoOps (one wait each) directly before the instruction."""
    nop_id = 0
    for fn in nc.m.functions:
        for blk in fn.blocks:
            out = []
            for inst in blk.instructions:
                if type(inst).__name__ != "InstNoOp":
                    si = inst.sync_info
                    waits = list(si.on_wait or []) if si is not None else []
                    if len(waits) > 1:
                        for w in waits[1:]:
                            nop = mybir.InstNoOp(
                                name=f"nopw-{nop_id}",
                                engine=inst.engine,
                                ins=[],
                                outs=[],
                                sync_info=mybir.SyncInfo(on_wait=[w], on_update=[]),
                            )
                            nop_id += 1
                            out.append(nop)
                        si.on_wait = waits[:1]
                out.append(inst)
            blk.instructions[:] = out


def build_nc(B, T, D, n_cores, legalize=True):
    nj = D // P  # contraction tiles for projections
    n_win = T // WIN  # q windows per batch
    n_qt = T // P  # token tiles per batch
    tpw = n_qt // n_win  # token tiles per window
    M = B * T

    nc = bass.Bass("TRN2", target_bir_lowering=False, debug=False, num_devices=n_cores)

    xt = nc.dram_tensor("xt", [D, M], BF, kind="ExternalInput").ap()
    wq = nc.dram_tensor("wq", [P, D], BF, kind="ExternalInput").ap()
    wk = nc.dram_tensor("wk", [P, D], BF, kind="ExternalInput").ap()
    wv = nc.dram_tensor("wv", [P, D], BF, kind="ExternalInput").ap()
    wo = nc.dram_tensor("wo", [P, D], BF, kind="ExternalInput").ap()
    bq = nc.dram_tensor("bq", [P, 1], F32, kind="ExternalInput").ap()
    bk = nc.dram_tensor("bk", [P, 1], F32, kind="ExternalInput").ap()
    msk = nc.dram_tensor("msk", [P, P], BF, kind="ExternalInput").ap()
    y = nc.dram_tensor("y", [M, D], BF, kind="ExternalOutput").ap()

    with tile.TileContext(nc) as tc, ExitStack() as ctx:
        const = ctx.enter_context(tc.tile_pool(name="const", bufs=1))
        xbp = ctx.enter_context(tc.tile_pool(name="xb", bufs=2))
        qkp = ctx.enter_context(tc.tile_pool(name="qk", bufs=2))
        vp = ctx.enter_context(tc.tile_pool(name="vaug", bufs=2))
        atp = ctx.enter_context(tc.tile_pool(name="attnT", bufs=8))
        aop = ctx.enter_context(tc.tile_pool(name="aoT", bufs=2))
        rcp = ctx.enter_context(tc.tile_pool(name="rc", bufs=2))
        psbp = ctx.enter_context(tc.tile_pool(name="psbs", bufs=2))
        pvrp = ctx.enter_context(tc.tile_pool(name="pvraw", bufs=4))
        yp = ctx.enter_context(tc.tile_pool(name="ysb", bufs=6))
        ps_sc = ctx.enter_context(tc.tile_pool(name="ps_sc", bufs=2, space="PSUM"))
        ps_pv = ctx.enter_context(tc.tile_pool(name="ps_pv", bufs=2, space="PSUM"))
        ps_proj = ctx.enter_context(tc.tile_pool(name="ps_proj", bufs=2, space="PSUM"))

        wq_s = const.tile([P, D], BF, tag="wq")
        wk_s = const.tile([P, D], BF, tag="wk")
        wv_s = const.tile([P, D], BF, tag="wv")
        wo_s = const.tile([P, D], BF, tag="wo")
        bq_s = const.tile([P, 1], F32, tag="bq")
        bk_s = const.tile([P, 1], F32, tag="bk")
        msk_s = const.tile([P, P], BF, tag="msk")
        warm_s = const.tile([P, WIN], BF, tag="warm")
        ones_s = const.tile([P, HD], BF, tag="ones")
        nc.vector.memset(warm_s[:, :], 1.0)
        nc.vector.memset(ones_s[:, :], 1.0)

        # x arrives window-major: one DMA per (batch, window) covering all nj
        # feature tiles, so window w's projections unblock after ~1MB.
        xbs = {}
        for b in range(B):
            xbs[b] = xbp.tile([P, n_win * nj * WIN], BF, tag="xb", name=f"xb{b}")

        def x_dma(b, w):
            src = xt[:, b * T + w * WIN : b * T + (w + 1) * WIN].rearrange(
                "(j p) c -> p j c", p=P
            )
            dst = xbs[b][:, w * nj * WIN : (w + 1) * nj * WIN].rearrange(
                "p (j c) -> p j c", j=nj
            )
            nc.sync.dma_start(dst, src)

        # q/k weights + first x window first; everything else behind them.
        nc.scalar.dma_start(wq_s[:, :], wq[:, :])
        x_dma(0, 0)
        nc.scalar.dma_start(wk_s[:, :], wk[:, :])
        nc.scalar.dma_start(bq_s[:, :], bq[:, :])
        nc.scalar.dma_start(bk_s[:, :], bk[:, :])
        x_dma(0, 1)
        nc.scalar.dma_start(wv_s[:, :], wv[:, :])
        nc.scalar.dma_start(msk_s[:, :], msk[:, :])
        x_dma(0, 2)
        x_dma(0, 3)
        nc.scalar.dma_start(wo_s[:, :], wo[:, :])
        for w in range(n_win):
            x_dma(1, w)

        # PE warm-up under the first x DMA's shadow.
        psw = ps_proj.tile([P, WIN], F32, tag="proj")
        for i in range(N_WARM):
            nc.tensor.matmul(
                psw[:, :], warm_s[:, 0:P], warm_s[:, :], start=True, stop=True
            )

        st = {}

        def xsl(b, w, j, off=0, n=WIN):
            base = (w * nj + j) * WIN + off
            return xbs[b][:, base : base + n]

        def alloc_batch(b):
            st[b] = {
                "qt": qkp.tile([P, T], BF, tag="qt", name=f"qt{b}"),
                "kt": qkp.tile([P, T], BF, tag="kt", name=f"kt{b}"),
                "vaug": vp.tile([P, n_qt * VW], BF, tag="vaug", name=f"vaug{b}"),
                "aoT": aop.tile([P, T], BF, tag="aoT", name=f"aoT{b}"),
                "vready": False,
            }

        def proj_chunk(b, w, which):
            s = st[b]
            ws = w * WIN
            w_s, b_s, dst = (
                (wq_s, bq_s, s["qt"]) if which == "q" else (wk_s, bk_s, s["kt"])
            )
            psp = ps_proj.tile([P, WIN], F32, tag="proj", name=f"ps{which}{b}_{w}")
            for j in range(nj):
                nc.tensor.matmul(
                    psp[:, :],
                    w_s[:, j * P : (j + 1) * P],
                    xsl(b, w, j),
                    start=(j == 0),
                    stop=(j == nj - 1),
                )
            # evac both heads in one DVE op, bias folded in (per-partition)
            nc.vector.tensor_scalar_add(
                dst[:, ws : ws + WIN], psp[:, :], b_s[:, 0:1]
            )

        def v_tile(b, t):
            s = st[b]
            if not s["vready"]:
                va4 = s["vaug"].rearrange("p (t s c) -> p t s c", s=HC, c=VB)
                nc.vector.memset(va4[:, :, :, HD : HD + 1], 1.0)  # ones col
                s["vready"] = True
            w, sub = t // tpw, t % tpw
            psv = ps_proj.tile([P, P], F32, tag="proj", name=f"psv{b}_{t}")
            for j in range(nj):
                nc.tensor.matmul(
                    psv[:, :],
                    xsl(b, w, j, sub * P, P),
                    wv_s[:, j * P : (j + 1) * P],
                    start=(j == 0),
                    stop=(j == nj - 1),
                )
            va4 = s["vaug"].rearrange("p (t s c) -> p t s c", s=HC, c=VB)
            nc.vector.tensor_copy(
                va4[:, t, :, 0:HD], psv[:, :].rearrange("p (s c) -> p s c", s=HC)
            )

        def qkv_thunks(b, w):
            th = [
                lambda b=b, w=w: proj_chunk(b, w, "q"),
                lambda b=b, w=w: proj_chunk(b, w, "k"),
            ]
            for t in range(w * tpw, (w + 1) * tpw):
                th.append(lambda b=b, t=t: v_tile(b, t))
            return th

        # out-proj PSUM evacuation round-robins across the three non-PE
        # compute engines so no single engine's backlog stalls the PE.
        evac_rr = [0]

        def outproj_tile(b, t):
            toff = b * T
            aoT = st[b]["aoT"]
            ysb = yp.tile([P, D], BF, tag="ysb", name=f"ysb{b}_{t}")
            for ui, u0 in enumerate(range(0, D, WIN)):
                psy = ps_proj.tile([P, WIN], F32, tag="proj", name=f"psy{b}_{t}_{ui}")
                nc.tensor.matmul(
                    psy[:, :],
                    aoT[:, t * P : (t + 1) * P],
                    wo_s[:, u0 : u0 + WIN],
                    start=True,
                    stop=True,
                )
                if tail_mode[0]:
                    # attention is over: DVE and Act are both idle; alternate
                    # so the two halves evacuate in parallel.
                    if ui == 0:
                        nc.scalar.copy(ysb[:, u0 : u0 + WIN], psy[:, :])
                    else:
                        nc.vector.tensor_copy(ysb[:, u0 : u0 + WIN], psy[:, :])
                else:
                    # DVE-only during attention: the Activation engine must
                    # stay dedicated to the exp stream that paces PV.
                    nc.vector.tensor_copy(ysb[:, u0 : u0 + WIN], psy[:, :])
            if tail_mode[0]:
                # spread tail DMA issue across idle engine queues
                r = evac_rr[0] = (evac_rr[0] + 1) % 3
                eng = (nc.sync, nc.gpsimd, nc.sync)[r]
                eng.dma_start(y[toff + t * P : toff + (t + 1) * P, :], ysb[:, :])
            else:
                nc.sync.dma_start(y[toff + t * P : toff + (t + 1) * P, :], ysb[:, :])

        # Global filler deque of (deadline_key, thunk).  Deadline keys are
        # global window indices (b*n_win+w) for thunks that MUST trace before
        # that attention window; soft thunks (out-proj) use 99.
        dq = []
        tail_mode = [False]
        # Filler reserve: keep ~2 thunks per remaining attention window so
        # the late windows (whose own out-proj cannot exist yet) still have
        # PE filler work -- otherwise the HAM clock gate halves the clock
        # right when the serial tail begins.
        reserve = [0]

        def pop_fill(n=1):
            if len(dq) > reserve[0] + 6:
                n += 1
            for _ in range(n):
                if len(dq) > reserve[0]:
                    dq.pop(0)[1]()

        def force_drain(gwi):
            # pop only entries whose deadline is due, preserving the relative
            # order of soft thunks (aoT evac stays ahead of its out-proj).
            i = 0
            while i < len(dq):
                if dq[i][0] <= gwi:
                    dq.pop(i)[1]()
                else:
                    i += 1

        def recip_phase(b, pspv, ws):
            # 1/d = exp(-ln(d)) on the Activation engine (InstReciprocal on
            # DVE costs ~4us/call and custom-DVE ISA ops fail this walrus
            # build's codegen).  Emitted at window end; the PE-side broadcast
            # is emitted a few filler thunks LATER so it never blocks the
            # in-order PE queue while this Act chain completes.
            lg = rcp.tile([P, WIN], F32, tag="lg", name=f"lg{b}_{ws}")
            rc2 = rcp.tile([P, WIN], BF, tag="rc", name=f"rc{b}_{ws}")
            LN = mybir.ActivationFunctionType.Ln
            for h in range(HC):
                nc.scalar.activation(
                    lg[h * HD : h * HD + 1, :], pspv[h][HD : HD + 1, :], LN
                )
                nc.scalar.activation(
                    rc2[h * HD : h * HD + 1, :],
                    lg[h * HD : h * HD + 1, :],
                    EXP,
                    scale=-1.0,
                )
            return rc2

        def aot_evac(b, pspv, rc2, ws):
            # normalized attn-out: aoT[h] = pspv[h][0:64] * (1/denominator).
            # 1/d sits on rows 0/64 of rc2 so the two K=1 broadcast matmuls
            # occupy disjoint 64x64 PE quadrants (dual-issued); the
            # row-replicated 1/d is staged to SBUF by the Activation engine
            # (one copy covers both heads) and the per-head multiplies run on
            # DVE (PSUM x SBUF -> SBUF bf16).
            psb = ps_proj.tile([P, WIN], F32, tag="proj", name=f"psb{b}_{ws}")
            for h in range(HC):
                nc.tensor.matmul(
                    psb[h * HD : (h + 1) * HD, :],
                    ones_s[h * HD : h * HD + 1, 0:HD],
                    rc2[h * HD : h * HD + 1, :],
                    start=True,
                    stop=True,
                )
            psbs = psbp.tile([P, WIN], BF, tag="psbs", name=f"psbs{b}_{ws}")
            nc.scalar.copy(psbs[:, :], psb[:, :])
            for h in range(HC):
                nc.vector.tensor_mul(
                    st[b]["aoT"][h * HD : (h + 1) * HD, ws : ws + WIN],
                    pspv[h][0:HD, :],
                    psbs[h * HD : (h + 1) * HD, :],
                )

        def attn_window(b, w):
            # Per k-tile j: both heads' K=64 score matmuls go back-to-back
            # into the two banks of one [128, 2*WIN] PSUM tile -> the PE
            # dual-issues them as a 64x128 row-tile pair.  One exp over a
            # strided [128, 2, N] AP covers both heads, so the pair of PV
            # matmuls (and the next window's score pair) release together.
            # PV for k-tile j is traced after the scores of j+1 so the PE
            # never waits on the exp; one filler thunk per k-tile keeps PE
            # demand (and the HAM clock) up.
            s = st[b]
            qt_s, kt_s, vaug = s["qt"], s["kt"], s["vaug"]
            ws = w * WIN
            njt = (ws + WIN) // P  # causal k tiles for this window
            pspv = [
                ps_pv.tile([VB, WIN], F32, tag="pv", name=f"pspv{b}_{w}_{_h}")
                for _h in range(HC)
            ]

            def flush_pv(at3, j, N, qoff):
                for h in range(HC):
                    nc.tensor.matmul(
                        pspv[h][:, qoff:WIN],
                        vaug[:, j * VW + h * VB : j * VW + h * VB + VB],
                        at3[:, h, 0:N],
                        start=(j == 0),
                        stop=(j == njt - 1),
                    )

            # PV lags the scores by TWO k-tiles: every cross-engine dep
            # (exp, mask, previous window's evac muls for the j==0 start)
            # is ~2us stale by the time the PE's in-order queue reaches the
            # PV matmul, so it never blocks the instructions behind it.
            pend = []
            for j in range(njt):
                qstart = max(ws, j * P)
                N = ws + WIN - qstart
                pss = ps_sc.tile([P, HC * WIN], F32, tag="sc", name=f"pss{b}_{w}_{j}")
                pss3 = pss.rearrange("p (h c) -> p h c", h=HC)
                if not dq:
                    # keep the HAM clock gate seeing PE activity when the
                    # filler stream runs dry
                    nc.tensor.matmul(
                        psw[:, :], warm_s[:, 0:P], warm_s[:, :], start=True, stop=True
                    )
                # the dual-issue pair: adjacent, disjoint row groups + banks
                for h in range(HC):
                    nc.tensor.matmul(
                        pss3[:, h, 0:N],
                        kt_s[h * HD : (h + 1) * HD, j * P : (j + 1) * P],
                        qt_s[h * HD : (h + 1) * HD, qstart : qstart + N],
                        start=True,
                        stop=True,
                    )
                at = atp.tile([P, HC * WIN], BF, tag="at", name=f"at{b}_{w}_{j}")
                at3 = at.rearrange("p (h c) -> p h c", h=HC)
                if N == WIN:
                    # contiguous 2D AP: the strided 3-dim form costs ~34% more
                    nc.scalar.activation(at[:, :], pss[:, :], EXP)
                else:
                    nc.scalar.activation(at3[:, :, 0:N], pss3[:, :, 0:N], EXP)
                if j * P >= ws:  # zero the upper triangle post-exp
                    for h in range(HC):
                        nc.gpsimd.tensor_mul(
                            at3[:, h, 0:P], at3[:, h, 0:P], msk_s[:, :]
                        )
                pend.append((at3, j, N, qstart - ws))
                if len(pend) > 2:
                    flush_pv(*pend.pop(0))
                    pop_fill(1)
            while pend:
                flush_pv(*pend.pop(0))
                pop_fill(1)
            # the Act-engine reciprocal chain overlaps the PE work below; the
            # PE-side broadcast only comes after it (in aot_evac), so the
            # in-order PE queue never stalls on the ~2us Act round-trip.
            # Coverage here is MANDATORY (ignore the reserve): without it the
            # broadcast matmul blocks the queue and the HAM clock halves.
            rc2 = recip_phase(b, pspv, ws)
            popped = 0
            while popped < 2 and dq:
                dq.pop(0)[1]()
                popped += 1
            for _ in range(0 if popped >= 2 else 4):
                nc.tensor.matmul(
                    psw[:, :], warm_s[:, 0:P], warm_s[:, :], start=True, stop=True
                )
            aot_evac(b, pspv, rc2, ws)
            # denominators: reciprocal (DVE) + row-broadcast (GpSimd) feed the
            # PV evacuation multiply; evacs are deadline thunks (must trace
            # before the next window reuses the PV PSUM banks), out-proj for
            # this window's tiles follows them in the deque as soft fillers.
            for t in range(w * tpw, (w + 1) * tpw):
                dq.append((99, lambda b=b, t=t: outproj_tile(b, t)))

        # ---- schedule ----
        # Only b0/w0's QKV runs eagerly so window-0 attention (and the Act
        # engine's exp stream) starts as early as possible; all remaining QKV
        # is deferred into the filler stream with per-window deadlines so the
        # whole kernel keeps enough PE demand to hold the HAM clock at 8/8.
        alloc_batch(0)
        alloc_batch(1)
        for f in qkv_thunks(0, 0):
            f()
        for w in range(1, n_win):
            dq.extend((w, f) for f in qkv_thunks(0, w))
        for w in (0, 1):
            dq.extend((n_win + w, f) for f in qkv_thunks(1, w))
        for b in range(B):
            for w in range(n_win):
                gwi = b * n_win + w
                reserve[0] = min(8, 2 * (B * n_win - gwi - 1))
                force_drain(gwi)
                attn_window(b, w)
                if (b, w) == (0, 2):
                    dq.extend((n_win + 2, f) for f in qkv_thunks(1, 2))
                if (b, w) == (0, 3):
                    dq.extend((n_win + 3, f) for f in qkv_thunks(1, 3))
        tail_mode[0] = True
        while dq:
            dq.pop(0)[1]()

    if legalize:
        _legalize_waits(nc)
    return nc


def make_in_maps(x, Wq, bq, Wk, bk, Wv, Wo, n_cores):
    x = np.asarray(x, dtype=np.float32)
    Bb, Tt, Dd = x.shape
    M = Bb * Tt
    xt = np.ascontiguousarray(x.reshape(M, Dd).T.astype(NPBF))
    mask = np.where(
        np.arange(P)[:, None] > np.arange(P)[None, :], 0.0, 1.0
    ).astype(NPBF)

    def wslice(W, c, scale=1.0):
        Wc = np.asarray(W, np.float32)[:, c * P : (c + 1) * P] * np.float32(scale)
        return np.ascontiguousarray(
            Wc.reshape(Dd // P, P, P).transpose(1, 0, 2).reshape(P, Dd).astype(NPBF)
        )

    qscale = 1.0 / np.sqrt(HD)
    in_maps = []
    for c in range(n_cores):
        cs = slice(c * P, (c + 1) * P)
        in_maps.append(
            {
                "xt": xt,
                "wq": wslice(Wq, c, qscale),
                "wk": wslice(Wk, c),
                "wv": wslice(Wv, c),
                "wo": np.ascontiguousarray(
                    np.asarray(Wo, np.float32)[cs, :].astype(NPBF)
                ),
                "bq": np.ascontiguousarray(
                    (np.asarray(bq, np.float32)[cs] * np.float32(qscale)).reshape(
                        P, 1
                    )
                ),
                "bk": np.ascontiguousarray(
                    np.asarray(bk, np.float32)[cs].reshape(P, 1)
                ),
                "msk": mask,
            }
        )
    return in_maps


_NC_CACHE = {}


def get_nc(B, T, D, n_cores):
    key = (B, T, D, n_cores)
    if key not in _NC_CACHE:
        _NC_CACHE[key] = build_nc(B, T, D, n_cores)
    return _NC_CACHE[key]


def kernel(**inputs):
    from concourse.bass_utils import run_bass_kernel_spmd

    x = np.asarray(inputs["x"], np.float32)
    Bb, Tt, Dd = x.shape
    n_cores = 8
    nc = get_nc(Bb, Tt, Dd, n_cores)
    in_maps = make_in_maps(
        x,
        inputs["Wq"],
        inputs["bq"],
        inputs["Wk"],
        inputs["bk"],
        inputs["Wv"],
        inputs["Wo"],
        n_cores,
    )
    res = run_bass_kernel_spmd(nc, in_maps, core_ids=list(range(n_cores)))
    y = np.zeros((Bb * Tt, Dd), dtype=np.float64)
    for r in res.results:
        y += r["y"].astype(np.float64)
    # V-bias is rank-1 through Wo; fold it (and bo) on the host.
    y += (
        np.asarray(inputs["bv"], np.float64) @ np.asarray(inputs["Wo"], np.float64)
        + np.asarray(inputs["bo"], np.float64)
    )[None, :]
    return y.reshape(Bb, Tt, Dd).astype(np.float32)
